# revision 109
# baseline (speedup 1.0000x reference)
"""Trainium2 Bass kernel for a 2-layer GCN (DeformationGNN).

Strategy (8 NeuronCores, SPMD), v2:
  - GCN layer restructured by linearity: scatter RAW node features
    first, apply the weight matrix per 128-dst window afterwards:
        segsum((h @ W)[src] * norm) + b == segsum(h[src] * norm) @ W + b
    so no replicated x @ W1 pre-pass exists; the layer-1 gather table is
    just x itself, quad-packed bf16 on the host.
  - Internal node relabeling n -> (n % 8) * OWN + n // 8 balances edges
    across cores; OWN = 13312 = 104 windows of 128, so every core has an
    identical window structure (SPMD) and packed table rows of each core
    form contiguous AllGather-compatible blocks.
  - Edges partitioned by dst owner into (window, chunk, parity) cells
    padded to 128-edge tiles; per-edge messages fetched with dma_gather
    (256B rows, int16 indices chunked at 32768), multi-window segments,
    gathers prefetched a few windows ahead and issued after each
    window's compute so Pool one-hot stragglers don't stall PE.
  - Scatter via PE matmul: lhsT = per-edge-tile one-hot built by
    iota==dst * norm (tensor_scalar); most one-hots on DVE, the tail
    tiles of each window on GpSimd (POOL_FRAC) to balance engines.
  - Layer 1 windows emit raw bf16 accumulators [32 feat, 128 dst] into
    1KB-contiguous staging; the exchange ships these pre-W1 accs (6.8MB
    total vs 13.6MB of h1) in three window-group AllGathers, each
    followed by a local pass computing h1 = relu(acc @ W1 + b1) for ALL
    nodes (cheap tiny matmuls + batched relu flushes, engines split
    ACT/DVE) into the pair-packed h1 table; local passes overlap the
    next collective (collectives barrier all Pool-dependent work).
  - Layer 2 windows apply W2 transposed (h2T = W2^T @ acc2) then Wf,
    biases folded in as ones-outer-product matmuls, final rows written
    batched per 8 windows.
"""

import sys

if '/opt/trn_rl_repo' not in sys.path:
    sys.path.insert(0, '/opt/trn_rl_repo')

import numpy as np

import concourse.bacc as bacc
import concourse.bass as bassmod
import concourse.mybir as mybir
import concourse.tile as tile
from concourse.bass_utils import run_bass_kernel_spmd

F32 = mybir.dt.float32
BF16 = mybir.dt.bfloat16
I16 = mybir.dt.int16

NC = 8            # cores
P = 128           # partitions / edge-tile size / window size
N = 100000        # real nodes
OWN = 13312       # padded nodes per core (104 windows)
NP_ = NC * OWN    # padded node count
NW = 104          # windows per core
CHUNK = 32768     # int16 index range per gather chunk
WB = 1            # windows per gather segment block
IN_DIM, HID, OUT_DIM = 32, 64, 3
NXR = NP_ // 4    # quad-packed x rows (26624)
NHR = NP_ // 2    # pair-packed h1 rows (53248)
SHARD = OWN // 2  # h1 rows per core (6656)
POOL_FRAC = 7     # fraction of one-hots built on GpSimd (1/PF)


def _cdiv(a, b):
    return (a + b - 1) // b


# ----------------------------------------------------------------- host prep


def _pack_stream(wn, ck, par, idx16, wloc, w, n_ck, n_par):
    """Sort edges into (window, chunk, parity) cells padded to 128-edge
    tiles; identical tile structure across cores (max over cores).
    Returns per-core streams + tile/segment metadata."""
    counts = np.zeros((NC, NW, n_ck, n_par), np.int64)
    for c in range(NC):
        np.add.at(counts[c], (wn[c], ck[c], par[c]), 1)
    tiles_cell = _cdiv(counts.max(axis=0), P)          # [NW, n_ck, n_par]

    # tile layout: for each wblock, for each ck, windows asc, par asc
    cells = {}                                          # (wn,ck,par)->(t0,nt)
    segs = []                                           # (ck, t0, nt)
    win_tiles = [[] for _ in range(NW)]                 # per wn: (ck,par,t0,nt)
    pos = 0
    for wb0 in range(0, NW, WB):
        for c_k in range(n_ck):
            s0 = pos
            for w_n in range(wb0, min(wb0 + WB, NW)):
                for p_r in range(n_par):
                    nt = int(tiles_cell[w_n, c_k, p_r])
                    if nt:
                        cells[(w_n, c_k, p_r)] = (pos, nt)
                        win_tiles[w_n].append((c_k, p_r, pos, nt))
                        pos += nt
            if pos > s0:
                segs.append((c_k, s0, pos - s0))
    T = pos
    segmax = max((nt for (_, _, nt) in segs), default=1)

    idx_arr = np.zeros((NC, T * P), np.int16)
    dl_arr = np.full((NC, T * P), -1.0, np.float32)
    ew_arr = np.zeros((NC, T * P), np.float32)
    for c in range(NC):
        key = (wn[c] * n_ck + ck[c]) * n_par + par[c]
        order = np.argsort(key, kind='stable')
        key_s = key[order]
        idx_s, wl_s, w_s = idx16[c][order], wloc[c][order], w[c][order]
        bounds = np.searchsorted(key_s, np.arange(NW * n_ck * n_par + 1))
        for (w_n, c_k, p_r), (t0, nt) in cells.items():
            k = (w_n * n_ck + c_k) * n_par + p_r
            lo, hi = bounds[k], bounds[k + 1]
            p0 = t0 * P
            idx_arr[c, p0:p0 + hi - lo] = idx_s[lo:hi]
            dl_arr[c, p0:p0 + hi - lo] = wl_s[lo:hi]
            ew_arr[c, p0:p0 + hi - lo] = w_s[lo:hi]

    def wrap(a):  # flat edge i -> [i%16, i//16], replicated in 8 stripes
        wr = a.reshape(-1, 16).T
        out = np.zeros((P, wr.shape[1]), np.int16)
        for g in range(8):
            out[16 * g:16 * g + 16] = wr
        return out

    streams = []
    for c in range(NC):
        streams.append({
            'idx': wrap(idx_arr[c]),
            'dl': np.ascontiguousarray(dl_arr[c].reshape(T, P).T),
            'ew': np.ascontiguousarray(ew_arr[c].reshape(T, P).T),
        })
    meta = dict(T=T, segs=segs, win_tiles=win_tiles, segmax=segmax,
                n_ck=n_ck, n_par=n_par)
    return streams, meta


def _prep(x, edge_index, edge_weight):
    x = np.asarray(x, np.float32)
    src_o = np.concatenate([np.asarray(edge_index[0], np.int64),
                            np.arange(N, dtype=np.int64)])
    dst_o = np.concatenate([np.asarray(edge_index[1], np.int64),
                            np.arange(N, dtype=np.int64)])
    w = np.concatenate([np.asarray(edge_weight, np.float64),
                        np.ones(N, np.float64)])

    # GCN symmetric normalization (original ids; value is perm-invariant)
    deg = np.zeros(N, np.float64)
    np.add.at(deg, dst_o, w)
    dis = np.where(deg > 0, 1.0 / np.sqrt(np.where(deg > 0, deg, 1.0)), 0.0)
    norm = (dis[src_o] * w * dis[dst_o]).astype(np.float32)

    # balanced internal relabeling
    src = (src_o % NC) * OWN + src_o // NC
    dst = (dst_o % NC) * OWN + dst_o // NC

    owner = dst // OWN
    dl = dst - owner * OWN
    gt = src >> 7
    sp = src & 127
    j = gt & 7
    row1 = ((gt >> 3) << 8) | (sp << 1) | (j >> 2)      # quad-packed x row
    par1 = j & 3
    row2 = ((gt >> 3) << 9) | (sp << 2) | (j >> 1)      # pair-packed h1 row
    par2 = j & 1
    ck2 = (row2 >> 15).astype(np.int64)
    idx2 = (row2 & (CHUNK - 1)).astype(np.int16)

    by = [owner == c for c in range(NC)]

    def per_core(a):
        return [a[m] for m in by]

    wn_c = per_core(dl >> 7)
    wl_c = per_core((dl & 127).astype(np.float32))
    w_c = per_core(norm)
    s1, m1 = _pack_stream(wn_c, [np.zeros_like(v) for v in wn_c],
                          per_core(par1), per_core(row1.astype(np.int16)),
                          wl_c, w_c, 1, 4)
    s2, m2 = _pack_stream(wn_c, per_core(ck2), per_core(par2),
                          per_core(idx2), wl_c, w_c, 2, 2)

    # quad-packed bf16 x table (internal order, fake nodes zero)
    import ml_dtypes
    x_int = np.zeros((NP_, IN_DIM), np.float32)
    n_orig = np.arange(N)
    x_int[(n_orig % NC) * OWN + n_orig // NC] = x
    xq = np.zeros((NXR, 4 * IN_DIM), ml_dtypes.bfloat16)
    r = np.arange(NXR)
    for q in range(4):
        nq = (((r >> 8) << 3) + ((r & 1) << 2) + q) * P + ((r >> 1) & 127)
        xq[:, q * IN_DIM:(q + 1) * IN_DIM] = x_int[nq]
    xq = xq.view(np.float32)

    return s1, m1, s2, m2, xq


# -------------------------------------------------------------- device build


def _build(m1, m2):
    T1, T2 = m1['T'], m2['T']
    nc = bacc.Bacc('TRN2', num_devices=NC)

    t_xq = nc.dram_tensor('xq', [NXR, 2 * IN_DIM], F32, kind='ExternalInput')
    t_idx1 = nc.dram_tensor('idx1', [P, T1 * 8], I16, kind='ExternalInput')
    t_dl1 = nc.dram_tensor('dl1', [P, T1], F32, kind='ExternalInput')
    t_ew1 = nc.dram_tensor('ew1', [P, T1], F32, kind='ExternalInput')
    t_idx2 = nc.dram_tensor('idx2', [P, T2 * 8], I16, kind='ExternalInput')
    t_dl2 = nc.dram_tensor('dl2', [P, T2], F32, kind='ExternalInput')
    t_ew2 = nc.dram_tensor('ew2', [P, T2], F32, kind='ExternalInput')
    t_iota = nc.dram_tensor('iota', [P, P], BF16, kind='ExternalInput')
    t_ones = nc.dram_tensor('ones1', [1, P], BF16, kind='ExternalInput')
    t_W1 = nc.dram_tensor('W1b', [IN_DIM, HID], BF16, kind='ExternalInput')
    t_W2 = nc.dram_tensor('W2b', [HID, HID], BF16, kind='ExternalInput')
    t_Wf = nc.dram_tensor('Wfb', [HID, OUT_DIM], BF16, kind='ExternalInput')
    t_b1 = nc.dram_tensor('b1r', [1, HID], BF16, kind='ExternalInput')
    t_b2 = nc.dram_tensor('b2r', [1, HID], BF16, kind='ExternalInput')
    t_bf = nc.dram_tensor('bfr', [1, OUT_DIM], BF16, kind='ExternalInput')
    t_out = nc.dram_tensor('out', [OWN, OUT_DIM], F32, kind='ExternalOutput')

    WGB = [0, 5, 9, 13]                # wg bounds of the exchange groups
    NG = 3
    NWR = 98                           # real (non-fake) windows per core
    GRW = [min(WGB[g + 1] * 8, NWR) - WGB[g] * 8 for g in range(NG)]
    GR = [GRW[g] * IN_DIM for g in range(NG)]
    t_ccas = [nc.dram_tensor(f'cca{g}', [GR[g], HID], F32, kind='Internal')
              for g in range(NG)]
    t_tabas = [nc.dram_tensor(f'taba{g}', [NC * GR[g], HID], F32,
                              kind='Internal', addr_space='Shared')
               for g in range(NG)]
    t_h1 = nc.dram_tensor('h1tab', [NHR, HID], F32, kind='Internal')
    groups = [list(range(NC))]

    from contextlib import ExitStack
    with tile.TileContext(nc) as tc, ExitStack() as es:
        cpool = es.enter_context(tc.tile_pool(name='const', bufs=1))
        spool = es.enter_context(tc.tile_pool(name='stream', bufs=1))
        msgp = es.enter_context(tc.tile_pool(name='msg', bufs=7))
        opool = es.enter_context(tc.tile_pool(name='onehot', bufs=72))
        apool = es.enter_context(tc.tile_pool(name='acc', bufs=4))
        hpool = es.enter_context(tc.tile_pool(name='hw', bufs=4))
        stgp = es.enter_context(tc.tile_pool(name='stg', bufs=3))

        # ---- constants
        iota_t = cpool.tile([P, P], BF16)
        nc.sync.dma_start(out=iota_t[:], in_=t_iota[:])
        ones_t = cpool.tile([1, P], BF16)
        nc.sync.dma_start(out=ones_t[:], in_=t_ones[:])
        W1t = cpool.tile([IN_DIM, HID], BF16)
        nc.sync.dma_start(out=W1t[:], in_=t_W1[:])
        W2t = cpool.tile([HID, HID], BF16)
        nc.sync.dma_start(out=W2t[:], in_=t_W2[:])
        Wft = cpool.tile([HID, OUT_DIM], BF16)
        nc.sync.dma_start(out=Wft[:], in_=t_Wf[:])
        b1t = cpool.tile([1, HID], BF16)
        nc.sync.dma_start(out=b1t[:], in_=t_b1[:])
        b2t = cpool.tile([1, HID], BF16)
        nc.sync.dma_start(out=b2t[:], in_=t_b2[:])
        bft = cpool.tile([1, OUT_DIM], BF16)
        nc.sync.dma_start(out=bft[:], in_=t_bf[:])

        # ---- streams (preloaded whole; spread across DMA queues)
        idx1b = spool.tile([P, T1 * 8], I16)
        nc.sync.dma_start(out=idx1b[:], in_=t_idx1[:])
        dl1b = spool.tile([P, T1], F32)
        nc.scalar.dma_start(out=dl1b[:], in_=t_dl1[:])
        ew1b = spool.tile([P, T1], F32)
        nc.scalar.dma_start(out=ew1b[:], in_=t_ew1[:])
        idx2b = spool.tile([P, T2 * 8], I16)
        nc.scalar.dma_start(out=idx2b[:], in_=t_idx2[:])
        dl2b = spool.tile([P, T2], F32)
        nc.scalar.dma_start(out=dl2b[:], in_=t_dl2[:])
        ew2b = spool.tile([P, T2], F32)
        nc.scalar.dma_start(out=ew2b[:], in_=t_ew2[:])

        tile_no = [0]

        def edge_pass(meta, table_aps, idxb, dlb, ewb, feat, win_cb, wr_cb,
                      psh_bufs=3, pool_frac=POOL_FRAC, psa_bufs=3):
            """Window-major scatter pass.

            table_aps: per-chunk dram APs (gather sources), rows of
            4*feat//... packed bf16 viewed f32 (elem 256B).
            win_cb(wn, acc_sb): consume [feat, 128] bf16 scatter result.
            wr_cb(wg): flush staging for 8-window group wg."""
            segs, win_tiles = meta['segs'], meta['win_tiles']
            segmax = meta['segmax']
            live = {}
            from contextlib import ExitStack as _ES
            pes = _ES()
            psa = pes.enter_context(
                tc.tile_pool(name=f'psa{feat}', bufs=psa_bufs, space='PSUM'))
            psh = pes.enter_context(
                tc.tile_pool(name=f'psh{feat}', bufs=psh_bufs, space='PSUM'))

            def issue_gather(si):
                c_k, t0, nt = segs[si]
                msg = msgp.tile([P, segmax * 64], F32, tag=f'msg{feat}')
                mv = msg[:].rearrange('p (t e) -> p t e', e=64)
                nc.gpsimd.dma_gather(
                    out_ap=mv[:, :nt, :],
                    in_ap=table_aps[c_k],
                    idxs_ap=idxb[:, t0 * 8:(t0 + nt) * 8],
                    num_idxs=nt * P, num_idxs_reg=nt * P,
                    elem_size=64, single_packet=False)
                live[si] = (msg, t0)

            # map each window -> set of segment indices it needs
            seg_of_t0 = {}
            for si, (c_k, t0, nt) in enumerate(segs):
                for t in range(t0, t0 + nt):
                    seg_of_t0[t] = si
            win_segs = []
            for w_n in range(NW):
                ss = sorted({seg_of_t0[t0] for (_, _, t0, nt)
                             in win_tiles[w_n]})
                win_segs.append(ss)

            issued = set()

            def ensure(si_list):
                for si in si_list:
                    if si not in issued:
                        issue_gather(si)
                        issued.add(si)

            la = max(WB, 5 - meta['n_ck'] * 2) + 1
            for w_n in range(min(la, NW)):
                ensure(win_segs[w_n])

            for w_n in range(NW):
                tl = win_tiles[w_n]
                ntot = sum(nt for (_, _, _, nt) in tl)
                if ntot == 0:
                    win_cb(w_n, None, psh)
                    if (w_n & 7) == 7:
                        wr_cb(w_n >> 3)
                    pf = w_n + la
                    if pf < NW:
                        ensure(win_segs[pf])
                    continue
                acc = psa.tile([feat, P], F32, tag=f'acc{feat}',
                               space='PSUM')
                n_pool = ntot // pool_frac
                k = 0
                for (c_k, p_r, t0, nt) in tl:
                    msg, s0 = live[seg_of_t0[t0]]
                    mb = msg[:].bitcast(BF16).rearrange(
                        'p (t e) -> p t e', e=128)
                    for tj in range(nt):
                        t = t0 + tj
                        O = opool.tile([P, P], BF16, tag='O')
                        eng = (nc.gpsimd if k >= ntot - n_pool
                               else nc.vector)
                        eng.tensor_scalar(
                            out=O[:], in0=iota_t[:],
                            scalar1=dlb[:, t:t + 1],
                            scalar2=ewb[:, t:t + 1],
                            op0=mybir.AluOpType.is_equal,
                            op1=mybir.AluOpType.mult)
                        nc.tensor.matmul(
                            out=acc[:],
                            lhsT=mb[:, t - s0, p_r * feat:(p_r + 1) * feat],
                            rhs=O[:],
                            start=(k == 0), stop=(k == ntot - 1))
                        k += 1
                win_cb(w_n, acc, psh)
                if (w_n & 7) == 7:
                    wr_cb(w_n >> 3)
                pf = w_n + la
                if pf < NW:
                    ensure(win_segs[pf])
            pes.close()

        # ===== layer 1: x -> raw acc windows (bf16), exchanged pre-W1
        stga = [None]

        def l1_win(w_n, acc, psh):
            if (w_n & 7) == 0:
                stga[0] = stgp.tile([IN_DIM, 8 * P], BF16, tag='sa',
                                    name='stga')
            j = w_n & 7
            dst = stga[0][:, j * P:(j + 1) * P]
            if acc is None:
                nc.vector.memset(dst, 0.0)
            else:
                nc.scalar.activation(dst, acc[:],
                                     mybir.ActivationFunctionType.Copy)

        def l1_wr(wg):
            g = next(i for i in range(NG) if wg < WGB[i + 1])
            r0 = (wg - WGB[g]) * 8
            w8 = min(8, GRW[g] - r0)
            dst = t_ccas[g][:, :].rearrange(
                '(p wn) e -> p wn e', p=IN_DIM)[:, r0:r0 + w8, :]
            nc.scalar.dma_start(
                out=dst,
                in_=stga[0][:].bitcast(F32).rearrange(
                    'p (w e) -> p w e', w=8)[:, :w8, :])
            if wg + 1 == WGB[g + 1]:
                nc.gpsimd.collective_compute(
                    'AllGather', mybir.AluOpType.bypass,
                    replica_groups=groups,
                    ins=[t_ccas[g][:, :]], outs=[t_tabas[g][:, :]])

        edge_pass(m1, [t_xq[:, :]], idx1b, dl1b, ew1b, IN_DIM, l1_win, l1_wr,
                  psa_bufs=6)

        # per-group: AllGather (SP stream: dispatches as soon as the
        # group's cca writes land, overlapping L1) then local pass
        # h1 = relu(acc @ W1 + b1) for that group's windows of all cores.
        with tc.tile_pool(name='lp', bufs=3) as lpool, \
             tc.tile_pool(name='stg1p', bufs=3) as stg1p, \
             tc.tile_pool(name='psg', bufs=2, space='PSUM') as psg:
            relu_no = 0
            for g in range(NG):
                ngw = WGB[g + 1] - WGB[g]
                nwn = GRW[g]
                for c2 in range(NC):
                    slab = lpool.tile([IN_DIM, 7 * 8 * HID], F32, tag='slab')
                    srcv = t_tabas[g][c2 * GR[g]:(c2 + 1) * GR[g], :
                                      ].rearrange('(p wn) e -> p (wn e)',
                                                  p=IN_DIM)
                    nc.sync.dma_start(out=slab[:, :nwn * HID], in_=srcv)
                    sb = slab[:].bitcast(BF16)
                    for wi in range(ngw):
                        if (wi & 3) == 0:
                            stgl = stg1p.tile([P, 4 * 8 * HID], BF16,
                                              tag='sg', name='stgl')
                            b0 = wi
                        nw = min(8, nwn - wi * 8)
                        ps = psg.tile([P, 8 * HID], F32, tag='ps',
                                      space='PSUM')
                        for w in range(nw):
                            wc = wi * 8 + w
                            nc.tensor.matmul(
                                out=ps[:, w * HID:(w + 1) * HID],
                                lhsT=ones_t[:], rhs=b1t[:],
                                start=True, stop=False)
                            nc.tensor.matmul(
                                out=ps[:, w * HID:(w + 1) * HID],
                                lhsT=sb[:, wc * P:(wc + 1) * P],
                                rhs=W1t[:], start=False, stop=True)
                        dstsl = stgl[:, (wi - b0) * 512:
                                     (wi - b0) * 512 + nw * HID]
                        if g == 0:
                            nc.scalar.activation(
                                dstsl, ps[:, :nw * HID],
                                mybir.ActivationFunctionType.Relu)
                        else:
                            nc.vector.tensor_scalar(
                                out=dstsl, in0=ps[:, :nw * HID], scalar1=0.0,
                                scalar2=None, op0=mybir.AluOpType.max)
                        relu_no += 1
                        if wi == ngw - 1 or (wi & 3) == 3:
                            nb = wi - b0 + 1
                            nfull = nb - (1 if nw < 8 else 0)
                            g0 = 13 * c2 + WGB[g] + b0
                            if nfull:
                                dstw = t_h1[g0 * 512:(g0 + nfull) * 512, :
                                            ].rearrange(
                                    '(b p q) e -> p b (q e)', p=P, b=nfull)
                                nc.scalar.dma_start(
                                    out=dstw,
                                    in_=stgl[:, :nfull * 512].bitcast(F32
                                        ).rearrange('p (b x) -> p b x',
                                                    b=nfull))
                            if nw < 8:
                                nq = nw // 2
                                dstp = t_h1[(g0 + nfull) * 512:
                                            (g0 + nfull + 1) * 512, :
                                            ].rearrange('(p q) e -> p q e',
                                                        p=P)[:, :nq, :]
                                srcp = stgl[:, nfull * 512:
                                            nfull * 512 + nw * HID
                                            ].bitcast(F32).rearrange(
                                    'p (b x) -> p b x', b=nq)
                                nc.scalar.dma_start(out=dstp, in_=srcp)

        # ================= layer 2: h1 -> out rows
        stg3 = [None]

        def l2_win(w_n, acc, psh):
            if (w_n & 7) == 0:
                stg3[0] = stgp.tile([P, 8 * OUT_DIM], F32, tag='s3', name='stg3')
            acc_sb = apool.tile([HID, P], BF16, tag='a2')
            if acc is None:
                nc.vector.memset(acc_sb[:], 0.0)
            else:
                nc.scalar.activation(acc_sb[:], acc[:],
                                     mybir.ActivationFunctionType.Copy)
            hp = psh.tile([HID, P], F32, tag='h2', space='PSUM')
            nc.tensor.matmul(out=hp[:], lhsT=b2t[:], rhs=ones_t[:],
                             start=True, stop=False)
            nc.tensor.matmul(out=hp[:], lhsT=W2t[:], rhs=acc_sb[:],
                             start=False, stop=True)
            h2T = hpool.tile([HID, P], BF16, tag='h2T')
            nc.scalar.activation(h2T[:], hp[:],
                                 mybir.ActivationFunctionType.Relu)
            op = psh.tile([P, OUT_DIM], F32, tag='o3', space='PSUM')
            nc.tensor.matmul(out=op[:], lhsT=ones_t[:], rhs=bft[:],
                             start=True, stop=False)
            nc.tensor.matmul(out=op[:], lhsT=h2T[:], rhs=Wft[:],
                             start=False, stop=True)
            j = w_n & 7
            nc.scalar.activation(
                stg3[0][:, j * OUT_DIM:(j + 1) * OUT_DIM], op[:],
                mybir.ActivationFunctionType.Copy)

        def l2_wr(wg):
            dst = t_out[wg * 1024:(wg + 1) * 1024, :].rearrange(
                '(w p) e -> p w e', p=P)
            nc.sync.dma_start(
                out=dst,
                in_=stg3[0][:].rearrange('p (w e) -> p w e', w=8))

        edge_pass(m2, [t_h1[0:CHUNK, :], t_h1[CHUNK:NHR, :]],
                  idx2b, dl2b, ew2b, HID, l2_win, l2_wr, psh_bufs=2,
                  psa_bufs=4)

    return nc


# ----------------------------------------------------------------- kernel()


def kernel(x, edge_index, edge_weight, W1, b1, W2, b2, Wf, bf,
           _sim=False):
    import ml_dtypes
    x = np.asarray(x)
    s1, m1, s2, m2, xq = _prep(x, np.asarray(edge_index),
                               np.asarray(edge_weight))

    iota_np = np.tile(np.arange(P, dtype=np.float32), (P, 1)).astype(
        ml_dtypes.bfloat16)
    common = {
        'xq': xq,
        'iota': iota_np,
        'ones1': np.ones((1, P), ml_dtypes.bfloat16),
        'W1b': np.asarray(W1, np.float32).astype(ml_dtypes.bfloat16),
        'W2b': np.asarray(W2, np.float32).astype(ml_dtypes.bfloat16),
        'Wfb': np.asarray(Wf, np.float32).astype(ml_dtypes.bfloat16),
        'b1r': np.asarray(b1, np.float32).reshape(1, HID).astype(
            ml_dtypes.bfloat16),
        'b2r': np.asarray(b2, np.float32).reshape(1, HID).astype(
            ml_dtypes.bfloat16),
        'bfr': np.asarray(bf, np.float32).reshape(1, OUT_DIM).astype(
            ml_dtypes.bfloat16),
    }
    in_maps = []
    for c in range(NC):
        d = dict(common)
        d.update({'idx1': s1[c]['idx'], 'dl1': s1[c]['dl'],
                  'ew1': s1[c]['ew'], 'idx2': s2[c]['idx'],
                  'dl2': s2[c]['dl'], 'ew2': s2[c]['ew']})
        in_maps.append(d)

    nc = _build(m1, m2)

    if _sim:
        from concourse.bass_interp import MultiCoreSim
        nc.compile()
        sim = MultiCoreSim(nc, num_cores=NC)
        for cid, core in sim.cores.items():
            for k, v in in_maps[cid].items():
                core.tensor(k)[:] = v
        sim.simulate()
        outs = [np.array(sim.cores[c].tensor('out')) for c in range(NC)]
        kernel.last_exec_ns = max(sim.cores[c].time for c in range(NC))
    else:
        nc.finalize()
        kernel.last_nc = nc
        res = run_bass_kernel_spmd(nc, in_maps, core_ids=list(range(NC)))
        kernel.last_exec_ns = res.exec_time_ns
        outs = [res.results[c]['out'] for c in range(NC)]

    outs = np.stack(outs)                                # [NC, OWN, 3]
    n = np.arange(N)
    return np.ascontiguousarray(outs[n % NC, n // NC, :])


# revision 113
# speedup vs baseline: 1.0021x; 1.0021x over previous
"""Trainium2 Bass kernel for a 2-layer GCN (DeformationGNN).

Strategy (8 NeuronCores, SPMD), v2:
  - GCN layer restructured by linearity: scatter RAW node features
    first, apply the weight matrix per 128-dst window afterwards:
        segsum((h @ W)[src] * norm) + b == segsum(h[src] * norm) @ W + b
    so no replicated x @ W1 pre-pass exists; the layer-1 gather table is
    just x itself, quad-packed bf16 on the host.
  - Internal node relabeling n -> (n % 8) * OWN + n // 8 balances edges
    across cores; OWN = 13312 = 104 windows of 128, so every core has an
    identical window structure (SPMD) and packed table rows of each core
    form contiguous AllGather-compatible blocks.
  - Edges partitioned by dst owner into (window, chunk, parity) cells
    padded to 128-edge tiles; per-edge messages fetched with dma_gather
    (256B rows, int16 indices chunked at 32768), multi-window segments,
    gathers prefetched a few windows ahead and issued after each
    window's compute so Pool one-hot stragglers don't stall PE.
  - Scatter via PE matmul: lhsT = per-edge-tile one-hot built by
    iota==dst * norm (tensor_scalar); most one-hots on DVE, the tail
    tiles of each window on GpSimd (POOL_FRAC) to balance engines.
  - Layer 1 windows emit raw bf16 accumulators [32 feat, 128 dst] into
    1KB-contiguous staging; the exchange ships these pre-W1 accs (6.8MB
    total vs 13.6MB of h1) in three window-group AllGathers, each
    followed by a local pass computing h1 = relu(acc @ W1 + b1) for ALL
    nodes (cheap tiny matmuls + batched relu flushes, engines split
    ACT/DVE) into the pair-packed h1 table; local passes overlap the
    next collective (collectives barrier all Pool-dependent work).
  - Layer 2 windows apply W2 transposed (h2T = W2^T @ acc2) then Wf,
    biases folded in as ones-outer-product matmuls, final rows written
    batched per 8 windows.
"""

import sys

if '/opt/trn_rl_repo' not in sys.path:
    sys.path.insert(0, '/opt/trn_rl_repo')

import numpy as np

import concourse.bacc as bacc
import concourse.bass as bassmod
import concourse.mybir as mybir
import concourse.tile as tile
from concourse.bass_utils import run_bass_kernel_spmd

F32 = mybir.dt.float32
BF16 = mybir.dt.bfloat16
I16 = mybir.dt.int16

NC = 8            # cores
P = 128           # partitions / edge-tile size / window size
N = 100000        # real nodes
OWN = 13312       # padded nodes per core (104 windows)
NP_ = NC * OWN    # padded node count
NW = 104          # windows per core
CHUNK = 32768     # int16 index range per gather chunk
WB = 1            # windows per gather segment block
IN_DIM, HID, OUT_DIM = 32, 64, 3
NXR = NP_ // 4    # quad-packed x rows (26624)
NHR = NP_ // 2    # pair-packed h1 rows (53248)
SHARD = OWN // 2  # h1 rows per core (6656)
POOL_FRAC = 7     # fraction of one-hots built on GpSimd (1/PF)


def _cdiv(a, b):
    return (a + b - 1) // b


# ----------------------------------------------------------------- host prep


def _pack_stream(wn, ck, par, idx16, wloc, w, n_ck, n_par):
    """Sort edges into (window, chunk, parity) cells padded to 128-edge
    tiles; identical tile structure across cores (max over cores).
    Returns per-core streams + tile/segment metadata."""
    counts = np.zeros((NC, NW, n_ck, n_par), np.int64)
    for c in range(NC):
        np.add.at(counts[c], (wn[c], ck[c], par[c]), 1)
    tiles_cell = _cdiv(counts.max(axis=0), P)          # [NW, n_ck, n_par]

    # tile layout: for each wblock, for each ck, windows asc, par asc
    cells = {}                                          # (wn,ck,par)->(t0,nt)
    segs = []                                           # (ck, t0, nt)
    win_tiles = [[] for _ in range(NW)]                 # per wn: (ck,par,t0,nt)
    pos = 0
    for wb0 in range(0, NW, WB):
        for c_k in range(n_ck):
            s0 = pos
            for w_n in range(wb0, min(wb0 + WB, NW)):
                for p_r in range(n_par):
                    nt = int(tiles_cell[w_n, c_k, p_r])
                    if nt:
                        cells[(w_n, c_k, p_r)] = (pos, nt)
                        win_tiles[w_n].append((c_k, p_r, pos, nt))
                        pos += nt
            if pos > s0:
                segs.append((c_k, s0, pos - s0))
    T = pos
    segmax = max((nt for (_, _, nt) in segs), default=1)

    idx_arr = np.zeros((NC, T * P), np.int16)
    dl_arr = np.full((NC, T * P), -1.0, np.float32)
    ew_arr = np.zeros((NC, T * P), np.float32)
    for c in range(NC):
        key = (wn[c] * n_ck + ck[c]) * n_par + par[c]
        order = np.argsort(key, kind='stable')
        key_s = key[order]
        idx_s, wl_s, w_s = idx16[c][order], wloc[c][order], w[c][order]
        bounds = np.searchsorted(key_s, np.arange(NW * n_ck * n_par + 1))
        for (w_n, c_k, p_r), (t0, nt) in cells.items():
            k = (w_n * n_ck + c_k) * n_par + p_r
            lo, hi = bounds[k], bounds[k + 1]
            p0 = t0 * P
            idx_arr[c, p0:p0 + hi - lo] = idx_s[lo:hi]
            dl_arr[c, p0:p0 + hi - lo] = wl_s[lo:hi]
            ew_arr[c, p0:p0 + hi - lo] = w_s[lo:hi]

    def wrap(a):  # flat edge i -> [i%16, i//16], replicated in 8 stripes
        wr = a.reshape(-1, 16).T
        out = np.zeros((P, wr.shape[1]), np.int16)
        for g in range(8):
            out[16 * g:16 * g + 16] = wr
        return out

    streams = []
    for c in range(NC):
        streams.append({
            'idx': wrap(idx_arr[c]),
            'dl': np.ascontiguousarray(dl_arr[c].reshape(T, P).T),
            'ew': np.ascontiguousarray(ew_arr[c].reshape(T, P).T),
        })
    meta = dict(T=T, segs=segs, win_tiles=win_tiles, segmax=segmax,
                n_ck=n_ck, n_par=n_par)
    return streams, meta


def _prep(x, edge_index, edge_weight):
    x = np.asarray(x, np.float32)
    src_o = np.concatenate([np.asarray(edge_index[0], np.int64),
                            np.arange(N, dtype=np.int64)])
    dst_o = np.concatenate([np.asarray(edge_index[1], np.int64),
                            np.arange(N, dtype=np.int64)])
    w = np.concatenate([np.asarray(edge_weight, np.float64),
                        np.ones(N, np.float64)])

    # GCN symmetric normalization (original ids; value is perm-invariant)
    deg = np.zeros(N, np.float64)
    np.add.at(deg, dst_o, w)
    dis = np.where(deg > 0, 1.0 / np.sqrt(np.where(deg > 0, deg, 1.0)), 0.0)
    norm = (dis[src_o] * w * dis[dst_o]).astype(np.float32)

    # balanced internal relabeling
    src = (src_o % NC) * OWN + src_o // NC
    dst = (dst_o % NC) * OWN + dst_o // NC

    owner = dst // OWN
    dl = dst - owner * OWN
    gt = src >> 7
    sp = src & 127
    j = gt & 7
    row1 = ((gt >> 3) << 8) | (sp << 1) | (j >> 2)      # quad-packed x row
    par1 = j & 3
    row2 = ((gt >> 3) << 9) | (sp << 2) | (j >> 1)      # pair-packed h1 row
    par2 = j & 1
    ck2 = (row2 >> 15).astype(np.int64)
    idx2 = (row2 & (CHUNK - 1)).astype(np.int16)

    by = [owner == c for c in range(NC)]

    def per_core(a):
        return [a[m] for m in by]

    wn_c = per_core(dl >> 7)
    wl_c = per_core((dl & 127).astype(np.float32))
    w_c = per_core(norm)
    s1, m1 = _pack_stream(wn_c, [np.zeros_like(v) for v in wn_c],
                          per_core(par1), per_core(row1.astype(np.int16)),
                          wl_c, w_c, 1, 4)
    s2, m2 = _pack_stream(wn_c, per_core(ck2), per_core(par2),
                          per_core(idx2), wl_c, w_c, 2, 2)

    # quad-packed bf16 x table (internal order, fake nodes zero)
    import ml_dtypes
    x_int = np.zeros((NP_, IN_DIM), np.float32)
    n_orig = np.arange(N)
    x_int[(n_orig % NC) * OWN + n_orig // NC] = x
    xq = np.zeros((NXR, 4 * IN_DIM), ml_dtypes.bfloat16)
    r = np.arange(NXR)
    for q in range(4):
        nq = (((r >> 8) << 3) + ((r & 1) << 2) + q) * P + ((r >> 1) & 127)
        xq[:, q * IN_DIM:(q + 1) * IN_DIM] = x_int[nq]
    xq = xq.view(np.float32)

    return s1, m1, s2, m2, xq


# -------------------------------------------------------------- device build


def _build(m1, m2):
    T1, T2 = m1['T'], m2['T']
    nc = bacc.Bacc('TRN2', num_devices=NC)

    t_xq = nc.dram_tensor('xq', [NXR, 2 * IN_DIM], F32, kind='ExternalInput')
    t_idx1 = nc.dram_tensor('idx1', [P, T1 * 8], I16, kind='ExternalInput')
    t_dl1 = nc.dram_tensor('dl1', [P, T1], F32, kind='ExternalInput')
    t_ew1 = nc.dram_tensor('ew1', [P, T1], F32, kind='ExternalInput')
    t_idx2 = nc.dram_tensor('idx2', [P, T2 * 8], I16, kind='ExternalInput')
    t_dl2 = nc.dram_tensor('dl2', [P, T2], F32, kind='ExternalInput')
    t_ew2 = nc.dram_tensor('ew2', [P, T2], F32, kind='ExternalInput')
    t_iota = nc.dram_tensor('iota', [P, P], BF16, kind='ExternalInput')
    t_ones = nc.dram_tensor('ones1', [1, P], BF16, kind='ExternalInput')
    t_W1 = nc.dram_tensor('W1b', [IN_DIM, HID], BF16, kind='ExternalInput')
    t_W2 = nc.dram_tensor('W2b', [HID, HID], BF16, kind='ExternalInput')
    t_Wf = nc.dram_tensor('Wfb', [HID, OUT_DIM], BF16, kind='ExternalInput')
    t_b1 = nc.dram_tensor('b1r', [1, HID], BF16, kind='ExternalInput')
    t_b2 = nc.dram_tensor('b2r', [1, HID], BF16, kind='ExternalInput')
    t_bf = nc.dram_tensor('bfr', [1, OUT_DIM], BF16, kind='ExternalInput')
    t_out = nc.dram_tensor('out', [OWN, OUT_DIM], F32, kind='ExternalOutput')

    WGB = [0, 5, 9, 13]                # wg bounds of the exchange groups
    NG = 3
    NWR = 98                           # real (non-fake) windows per core
    GRW = [min(WGB[g + 1] * 8, NWR) - WGB[g] * 8 for g in range(NG)]
    GR = [GRW[g] * IN_DIM for g in range(NG)]
    t_ccas = [nc.dram_tensor(f'cca{g}', [GR[g], HID], F32, kind='Internal')
              for g in range(NG)]
    t_tabas = [nc.dram_tensor(f'taba{g}', [NC * GR[g], HID], F32,
                              kind='Internal', addr_space='Shared')
               for g in range(NG)]
    t_h1 = nc.dram_tensor('h1tab', [NHR, HID], F32, kind='Internal')
    groups = [list(range(NC))]

    from contextlib import ExitStack
    with tile.TileContext(nc) as tc, ExitStack() as es:
        cpool = es.enter_context(tc.tile_pool(name='const', bufs=1))
        spool = es.enter_context(tc.tile_pool(name='stream', bufs=1))
        msgp = es.enter_context(tc.tile_pool(name='msg', bufs=7))
        opool = es.enter_context(tc.tile_pool(name='onehot', bufs=72))
        apool = es.enter_context(tc.tile_pool(name='acc', bufs=4))
        hpool = es.enter_context(tc.tile_pool(name='hw', bufs=4))
        stgp = es.enter_context(tc.tile_pool(name='stg', bufs=3))

        # ---- constants
        iota_t = cpool.tile([P, P], BF16)
        nc.sync.dma_start(out=iota_t[:], in_=t_iota[:])
        ones_t = cpool.tile([1, P], BF16)
        nc.sync.dma_start(out=ones_t[:], in_=t_ones[:])
        W1t = cpool.tile([IN_DIM, HID], BF16)
        nc.sync.dma_start(out=W1t[:], in_=t_W1[:])
        W2t = cpool.tile([HID, HID], BF16)
        nc.sync.dma_start(out=W2t[:], in_=t_W2[:])
        Wft = cpool.tile([HID, OUT_DIM], BF16)
        nc.sync.dma_start(out=Wft[:], in_=t_Wf[:])
        b1t = cpool.tile([1, HID], BF16)
        nc.sync.dma_start(out=b1t[:], in_=t_b1[:])
        b2t = cpool.tile([1, HID], BF16)
        nc.sync.dma_start(out=b2t[:], in_=t_b2[:])
        bft = cpool.tile([1, OUT_DIM], BF16)
        nc.sync.dma_start(out=bft[:], in_=t_bf[:])

        # ---- streams (preloaded whole; spread across DMA queues)
        idx1b = spool.tile([P, T1 * 8], I16)
        nc.sync.dma_start(out=idx1b[:], in_=t_idx1[:])
        dl1b = spool.tile([P, T1], F32)
        nc.scalar.dma_start(out=dl1b[:], in_=t_dl1[:])
        ew1b = spool.tile([P, T1], F32)
        nc.scalar.dma_start(out=ew1b[:], in_=t_ew1[:])
        idx2b = spool.tile([P, T2 * 8], I16)
        nc.scalar.dma_start(out=idx2b[:], in_=t_idx2[:])
        dl2b = spool.tile([P, T2], F32)
        nc.scalar.dma_start(out=dl2b[:], in_=t_dl2[:])
        ew2b = spool.tile([P, T2], F32)
        nc.scalar.dma_start(out=ew2b[:], in_=t_ew2[:])

        tile_no = [0]

        def edge_pass(meta, table_aps, idxb, dlb, ewb, feat, win_cb, wr_cb,
                      psh_bufs=3, pool_frac=POOL_FRAC, psa_bufs=3):
            """Window-major scatter pass.

            table_aps: per-chunk dram APs (gather sources), rows of
            4*feat//... packed bf16 viewed f32 (elem 256B).
            win_cb(wn, acc_sb): consume [feat, 128] bf16 scatter result.
            wr_cb(wg): flush staging for 8-window group wg."""
            segs, win_tiles = meta['segs'], meta['win_tiles']
            segmax = meta['segmax']
            live = {}
            from contextlib import ExitStack as _ES
            pes = _ES()
            psa = pes.enter_context(
                tc.tile_pool(name=f'psa{feat}', bufs=psa_bufs, space='PSUM'))
            psh = pes.enter_context(
                tc.tile_pool(name=f'psh{feat}', bufs=psh_bufs, space='PSUM'))

            def issue_gather(si):
                c_k, t0, nt = segs[si]
                msg = msgp.tile([P, segmax * 64], F32, tag=f'msg{feat}')
                mv = msg[:].rearrange('p (t e) -> p t e', e=64)
                nc.gpsimd.dma_gather(
                    out_ap=mv[:, :nt, :],
                    in_ap=table_aps[c_k],
                    idxs_ap=idxb[:, t0 * 8:(t0 + nt) * 8],
                    num_idxs=nt * P, num_idxs_reg=nt * P,
                    elem_size=64, single_packet=False)
                live[si] = (msg, t0)

            # map each window -> set of segment indices it needs
            seg_of_t0 = {}
            for si, (c_k, t0, nt) in enumerate(segs):
                for t in range(t0, t0 + nt):
                    seg_of_t0[t] = si
            win_segs = []
            for w_n in range(NW):
                ss = sorted({seg_of_t0[t0] for (_, _, t0, nt)
                             in win_tiles[w_n]})
                win_segs.append(ss)

            issued = set()

            def ensure(si_list):
                for si in si_list:
                    if si not in issued:
                        issue_gather(si)
                        issued.add(si)

            la = max(WB, 3 - meta['n_ck'] * 2)
            for w_n in range(min(la, NW)):
                ensure(win_segs[w_n])

            for w_n in range(NW):
                tl = win_tiles[w_n]
                ntot = sum(nt for (_, _, _, nt) in tl)
                if ntot == 0:
                    win_cb(w_n, None, psh)
                    if (w_n & 7) == 7:
                        wr_cb(w_n >> 3)
                    pf = w_n + la
                    if pf < NW:
                        ensure(win_segs[pf])
                    continue
                acc = psa.tile([feat, P], F32, tag=f'acc{feat}',
                               space='PSUM')
                n_pool = ntot // pool_frac
                k = 0
                for (c_k, p_r, t0, nt) in tl:
                    msg, s0 = live[seg_of_t0[t0]]
                    mb = msg[:].bitcast(BF16).rearrange(
                        'p (t e) -> p t e', e=128)
                    for tj in range(nt):
                        t = t0 + tj
                        O = opool.tile([P, P], BF16, tag='O')
                        eng = (nc.gpsimd if k >= ntot - n_pool
                               else nc.vector)
                        eng.tensor_scalar(
                            out=O[:], in0=iota_t[:],
                            scalar1=dlb[:, t:t + 1],
                            scalar2=ewb[:, t:t + 1],
                            op0=mybir.AluOpType.is_equal,
                            op1=mybir.AluOpType.mult)
                        nc.tensor.matmul(
                            out=acc[:],
                            lhsT=mb[:, t - s0, p_r * feat:(p_r + 1) * feat],
                            rhs=O[:],
                            start=(k == 0), stop=(k == ntot - 1))
                        k += 1
                win_cb(w_n, acc, psh)
                if (w_n & 7) == 7:
                    wr_cb(w_n >> 3)
                pf = w_n + la
                if pf < NW:
                    ensure(win_segs[pf])
            pes.close()

        # ===== layer 1: x -> raw acc windows (bf16), exchanged pre-W1
        stga = [None]

        def l1_win(w_n, acc, psh):
            if (w_n & 7) == 0:
                stga[0] = stgp.tile([IN_DIM, 8 * P], BF16, tag='sa',
                                    name='stga')
            j = w_n & 7
            dst = stga[0][:, j * P:(j + 1) * P]
            if acc is None:
                nc.vector.memset(dst, 0.0)
            else:
                nc.scalar.activation(dst, acc[:],
                                     mybir.ActivationFunctionType.Copy)

        def l1_wr(wg):
            g = next(i for i in range(NG) if wg < WGB[i + 1])
            r0 = (wg - WGB[g]) * 8
            w8 = min(8, GRW[g] - r0)
            dst = t_ccas[g][:, :].rearrange(
                '(p wn) e -> p wn e', p=IN_DIM)[:, r0:r0 + w8, :]
            nc.scalar.dma_start(
                out=dst,
                in_=stga[0][:].bitcast(F32).rearrange(
                    'p (w e) -> p w e', w=8)[:, :w8, :])
            if wg + 1 == WGB[g + 1]:
                nc.gpsimd.collective_compute(
                    'AllGather', mybir.AluOpType.bypass,
                    replica_groups=groups,
                    ins=[t_ccas[g][:, :]], outs=[t_tabas[g][:, :]])

        edge_pass(m1, [t_xq[:, :]], idx1b, dl1b, ew1b, IN_DIM, l1_win, l1_wr,
                  psa_bufs=6)

        # per-group: AllGather (SP stream: dispatches as soon as the
        # group's cca writes land, overlapping L1) then local pass
        # h1 = relu(acc @ W1 + b1) for that group's windows of all cores.
        with tc.tile_pool(name='lp', bufs=3) as lpool, \
             tc.tile_pool(name='stg1p', bufs=3) as stg1p, \
             tc.tile_pool(name='psg', bufs=2, space='PSUM') as psg:
            relu_no = 0
            for g in range(NG):
                ngw = WGB[g + 1] - WGB[g]
                nwn = GRW[g]
                for c2 in range(NC):
                    slab = lpool.tile([IN_DIM, 7 * 8 * HID], F32, tag='slab')
                    srcv = t_tabas[g][c2 * GR[g]:(c2 + 1) * GR[g], :
                                      ].rearrange('(p wn) e -> p (wn e)',
                                                  p=IN_DIM)
                    nc.sync.dma_start(out=slab[:, :nwn * HID], in_=srcv)
                    sb = slab[:].bitcast(BF16)
                    for wi in range(ngw):
                        if (wi & 3) == 0:
                            stgl = stg1p.tile([P, 4 * 8 * HID], BF16,
                                              tag='sg', name='stgl')
                            b0 = wi
                        nw = min(8, nwn - wi * 8)
                        ps = psg.tile([P, 8 * HID], F32, tag='ps',
                                      space='PSUM')
                        for w in range(nw):
                            wc = wi * 8 + w
                            nc.tensor.matmul(
                                out=ps[:, w * HID:(w + 1) * HID],
                                lhsT=ones_t[:], rhs=b1t[:],
                                start=True, stop=False)
                            nc.tensor.matmul(
                                out=ps[:, w * HID:(w + 1) * HID],
                                lhsT=sb[:, wc * P:(wc + 1) * P],
                                rhs=W1t[:], start=False, stop=True)
                        dstsl = stgl[:, (wi - b0) * 512:
                                     (wi - b0) * 512 + nw * HID]
                        if g == 0:
                            nc.scalar.activation(
                                dstsl, ps[:, :nw * HID],
                                mybir.ActivationFunctionType.Relu)
                        else:
                            nc.vector.tensor_scalar(
                                out=dstsl, in0=ps[:, :nw * HID], scalar1=0.0,
                                scalar2=None, op0=mybir.AluOpType.max)
                        relu_no += 1
                        if wi == ngw - 1 or (wi & 3) == 3:
                            nb = wi - b0 + 1
                            nfull = nb - (1 if nw < 8 else 0)
                            g0 = 13 * c2 + WGB[g] + b0
                            if nfull:
                                dstw = t_h1[g0 * 512:(g0 + nfull) * 512, :
                                            ].rearrange(
                                    '(b p q) e -> p b (q e)', p=P, b=nfull)
                                nc.scalar.dma_start(
                                    out=dstw,
                                    in_=stgl[:, :nfull * 512].bitcast(F32
                                        ).rearrange('p (b x) -> p b x',
                                                    b=nfull))
                            if nw < 8:
                                nq = nw // 2
                                dstp = t_h1[(g0 + nfull) * 512:
                                            (g0 + nfull + 1) * 512, :
                                            ].rearrange('(p q) e -> p q e',
                                                        p=P)[:, :nq, :]
                                srcp = stgl[:, nfull * 512:
                                            nfull * 512 + nw * HID
                                            ].bitcast(F32).rearrange(
                                    'p (b x) -> p b x', b=nq)
                                nc.scalar.dma_start(out=dstp, in_=srcp)

        # ================= layer 2: h1 -> out rows
        stg3 = [None]

        def l2_win(w_n, acc, psh):
            if (w_n & 7) == 0:
                stg3[0] = stgp.tile([P, 8 * OUT_DIM], F32, tag='s3', name='stg3')
            acc_sb = apool.tile([HID, P], BF16, tag='a2')
            if acc is None:
                nc.vector.memset(acc_sb[:], 0.0)
            else:
                nc.scalar.activation(acc_sb[:], acc[:],
                                     mybir.ActivationFunctionType.Copy)
            hp = psh.tile([HID, P], F32, tag='h2', space='PSUM')
            nc.tensor.matmul(out=hp[:], lhsT=b2t[:], rhs=ones_t[:],
                             start=True, stop=False)
            nc.tensor.matmul(out=hp[:], lhsT=W2t[:], rhs=acc_sb[:],
                             start=False, stop=True)
            h2T = hpool.tile([HID, P], BF16, tag='h2T')
            nc.scalar.activation(h2T[:], hp[:],
                                 mybir.ActivationFunctionType.Relu)
            op = psh.tile([P, OUT_DIM], F32, tag='o3', space='PSUM')
            nc.tensor.matmul(out=op[:], lhsT=ones_t[:], rhs=bft[:],
                             start=True, stop=False)
            nc.tensor.matmul(out=op[:], lhsT=h2T[:], rhs=Wft[:],
                             start=False, stop=True)
            j = w_n & 7
            nc.scalar.activation(
                stg3[0][:, j * OUT_DIM:(j + 1) * OUT_DIM], op[:],
                mybir.ActivationFunctionType.Copy)

        def l2_wr(wg):
            dst = t_out[wg * 1024:(wg + 1) * 1024, :].rearrange(
                '(w p) e -> p w e', p=P)
            nc.sync.dma_start(
                out=dst,
                in_=stg3[0][:].rearrange('p (w e) -> p w e', w=8))

        edge_pass(m2, [t_h1[0:CHUNK, :], t_h1[CHUNK:NHR, :]],
                  idx2b, dl2b, ew2b, HID, l2_win, l2_wr, psh_bufs=2,
                  psa_bufs=4)

    return nc


# ----------------------------------------------------------------- kernel()


def kernel(x, edge_index, edge_weight, W1, b1, W2, b2, Wf, bf,
           _sim=False):
    import ml_dtypes
    x = np.asarray(x)
    s1, m1, s2, m2, xq = _prep(x, np.asarray(edge_index),
                               np.asarray(edge_weight))

    iota_np = np.tile(np.arange(P, dtype=np.float32), (P, 1)).astype(
        ml_dtypes.bfloat16)
    common = {
        'xq': xq,
        'iota': iota_np,
        'ones1': np.ones((1, P), ml_dtypes.bfloat16),
        'W1b': np.asarray(W1, np.float32).astype(ml_dtypes.bfloat16),
        'W2b': np.asarray(W2, np.float32).astype(ml_dtypes.bfloat16),
        'Wfb': np.asarray(Wf, np.float32).astype(ml_dtypes.bfloat16),
        'b1r': np.asarray(b1, np.float32).reshape(1, HID).astype(
            ml_dtypes.bfloat16),
        'b2r': np.asarray(b2, np.float32).reshape(1, HID).astype(
            ml_dtypes.bfloat16),
        'bfr': np.asarray(bf, np.float32).reshape(1, OUT_DIM).astype(
            ml_dtypes.bfloat16),
    }
    in_maps = []
    for c in range(NC):
        d = dict(common)
        d.update({'idx1': s1[c]['idx'], 'dl1': s1[c]['dl'],
                  'ew1': s1[c]['ew'], 'idx2': s2[c]['idx'],
                  'dl2': s2[c]['dl'], 'ew2': s2[c]['ew']})
        in_maps.append(d)

    nc = _build(m1, m2)

    if _sim:
        from concourse.bass_interp import MultiCoreSim
        nc.compile()
        sim = MultiCoreSim(nc, num_cores=NC)
        for cid, core in sim.cores.items():
            for k, v in in_maps[cid].items():
                core.tensor(k)[:] = v
        sim.simulate()
        outs = [np.array(sim.cores[c].tensor('out')) for c in range(NC)]
        kernel.last_exec_ns = max(sim.cores[c].time for c in range(NC))
    else:
        nc.finalize()
        kernel.last_nc = nc
        res = run_bass_kernel_spmd(nc, in_maps, core_ids=list(range(NC)))
        kernel.last_exec_ns = res.exec_time_ns
        outs = [res.results[c]['out'] for c in range(NC)]

    outs = np.stack(outs)                                # [NC, OWN, 3]
    n = np.arange(N)
    return np.ascontiguousarray(outs[n % NC, n // NC, :])
